# revision 16
# baseline (speedup 1.0000x reference)
"""Trainium2 Bass kernel for nn_MetaDataTokens (dense_cnn).

Pure data-parallel over 8 NeuronCores: batch 16384 -> 2048 per core, all
parameters replicated. Per-core pipeline (per 512-example tile):

  one-hot slabs precomputed on host, DMA'd in (bf16)
  ms/xsum/G accumulated straight off the one-hot slabs (PE)
  r = rsqrt(ms/S + eps) (ACT sqrt + DVE recip, bf16)
  xg = DMA-gather of bf16 embedding rows into [d, (s,b)] layout
  per s: xn = xg*r (DVE bf16 2x); xd' = lp*conv(xn) (PE banded matmul);
         P = emb^T G-slice (PE); Pb = bf16(P) (ACT copy); gpre = Pb*r (DVE 2x);
         g = silu(gpre+gb) (ACT); h' = (xd'+lp*bsum)*g (DVE stt);
         sq = h'*h' (GPSIMD, SBUF only)
  o2 = sum_s h' (PE iden-accum), m2 = sum_s invlp_s^2*sq_s (PE scaled-iden)
  out = xsum + rsqrt(m2/S+eps)*o2, stored [d, b] bf16; host transposes.
"""

import sys

if "/opt/trn_rl_repo" not in sys.path:
    sys.path.insert(0, "/opt/trn_rl_repo")

from contextlib import ExitStack

import numpy as np

import concourse.bass as bass
import concourse.bacc as bacc
import concourse.mybir as mybir
import concourse.tile as tile
from concourse import library_config

AF = mybir.ActivationFunctionType
ALU = mybir.AluOpType
dt = mybir.dt
F32 = dt.float32
BF16 = dt.bfloat16

B, S, D, V = 16384, 20, 128, 12
NCORES = 8
BC = B // NCORES  # 2048 batch per core
BT = 512          # batch tile (one PSUM bank wide)
NT = BC // BT     # 4
NSLAB = 5         # one-hot slabs: 4 s-values x 32 partition rows each
NIC = (S * BT) // 16  # gather-index columns per tile (16-partition wrap)
EPS = 1e-5

USE_DMA_GATHER = False

WEIGHT_NAMES = [
    "emb", "stm", "stx", "stg", "emb4g", "kmat",
    "gbb", "lpbsb", "invlpb", "epsb", "zerob",
]
BF16_SET = {"emb", "stm", "stx", "stg", "emb4g", "kmat", "oh5", "ohp"}


def _derived(inputs):
    """Host-side preprocessing of the (tiny) parameter tensors."""
    f = np.float32
    emb = np.asarray(inputs["emb"], f)              # [12, 128]
    pre_w = np.asarray(inputs["pre_w"], f)          # [20]
    post_w = np.asarray(inputs["post_w"], f)        # [20]
    gate_w = np.asarray(inputs["gate_w"], f)        # [20, 20]
    gate_b = np.asarray(inputs["gate_b"], f)        # [20]
    lw = np.asarray(inputs["logit_w"], f)[0, :, 0]  # [20]
    bsum = (np.asarray(inputs["b0"], f) + np.asarray(inputs["b1"], f)
            + np.asarray(inputs["b2"], f))          # [20]

    lp = lw * post_w
    mx = float(np.abs(lp).max())
    floor = max(mx, 1e-30) * 1e-8
    lp_eff = np.where(np.abs(lp) < floor, np.where(lp < 0, -floor, floor), lp).astype(f)
    invlp = (1.0 / lp_eff).astype(f)

    # combined conv taps: Wc[s, o+4] for offsets o in [-4, 4]
    Wc = np.zeros((S, 9), f)
    for w_, dil in ((inputs["w0"], 1), (inputs["w1"], 2), (inputs["w2"], 4)):
        w_ = np.asarray(w_, f)
        for k in range(3):
            Wc[:, (k - 1) * dil + 4] += w_[:, 0, k]

    # banded conv matrices, lp_eff and pre_w folded in.
    # kmat[d_in, s*D + d_out] = lp_eff[s]*pre_w[s]*sum_o Wc[s,o]*[d_in == d_out+o]
    kmat = np.zeros((D, S * D), f)
    d_out = np.arange(D)
    for s in range(S):
        c0 = lp_eff[s] * pre_w[s]
        for o in range(-4, 5):
            cs = c0 * Wc[s, o + 4]
            if cs == 0.0:
                continue
            d_in = d_out + o
            valid = (d_in >= 0) & (d_in < D)
            kmat[d_in[valid], s * D + d_out[valid]] += cs

    # ms accumulation from ohP (rows 12s'+v): -> emb[v,:]^2, same both slabs
    e2q = (emb * emb).astype(f)
    stm = np.zeros((D, D), f)
    for sp in range(10):
        stm[12 * sp:12 * sp + V, :] = e2q

    # xsum accumulation from ohP: block p rows 12s'+v -> lw[10p+s']*emb[v,:]
    stx = np.zeros((D, 2 * D), f)
    for p in range(2):
        for sp in range(10):
            stx[12 * sp:12 * sp + V, p * D:(p + 1) * D] = lw[10 * p + sp] * emb

    # gate-coefficient stationaries from ohP: G-tile gi holds t = 4gi+q at
    # rows 32q+v.  stg block (gi, p): [12s'+v, 32q+v'] =
    # gwp[4gi+q, 10p+s'] * [v==v']
    gwp = gate_w * pre_w[None, :]  # [t, s]
    eye = np.eye(V, dtype=f)
    stg = np.zeros((D, NSLAB * 2 * D), f)
    for gi in range(NSLAB):
        for p in range(2):
            blk = np.zeros((D, D), f)
            for q in range(4):
                for sp in range(10):
                    blk[12 * sp:12 * sp + V, 32 * q:32 * q + V] = (
                        gwp[4 * gi + q, 10 * p + sp] * eye)
            stg[:, (gi * 2 + p) * D:(gi * 2 + p + 1) * D] = blk

    # emb replicated at the four 32-row bases (gateP stationary)
    emb4g = np.zeros((D, D), f)
    for q in range(4):
        emb4g[32 * q:32 * q + V, :] = emb

    rep = lambda v: np.repeat(np.asarray(v, f)[None, :], D, 0)
    return {
        "emb": emb, "stm": stm, "stx": stx, "stg": stg, "emb4g": emb4g,
        "kmat": kmat,
        "gbb": rep(gate_b), "lpbsb": rep(lp_eff * bsum),
        "invlpb": rep(invlp),
        "epsb": np.full((D, 1), EPS, f), "zerob": np.zeros((D, 1), f),
    }


def build_program(act_silu=True):
    """Build the per-core Bass program (same program for every core)."""
    nc = bacc.Bacc("TRN2", target_bir_lowering=False)
    gidx_d = nc.declare_dram_parameter("gidx", [16, NT * (S * BT) // 16], dt.int16,
                                       isOutput=False)
    oh_d = nc.declare_dram_parameter("oh5", [NSLAB * D, BC], BF16, isOutput=False)
    ohp_d = nc.declare_dram_parameter("ohp", [2 * D, BC], BF16, isOutput=False)
    shapes = {
        "emb": (V, D), "stm": (D, D), "stx": (D, 2 * D),
        "stg": (D, NSLAB * 2 * D), "emb4g": (D, D), "kmat": (D, S * D),
        "gbb": (D, S), "lpbsb": (D, S), "invlpb": (D, S), "epsb": (D, 1), "zerob": (D, 1),
    }
    wdt = {n: (BF16 if n in BF16_SET else F32) for n in WEIGHT_NAMES}
    dram = {n: nc.declare_dram_parameter(n, list(shapes[n]), wdt[n], isOutput=False)
            for n in WEIGHT_NAMES}
    out_d = nc.declare_dram_parameter("out", [D, BC], BF16, isOutput=True)

    AFg = AF.Silu if act_silu else AF.Sigmoid

    with tile.TileContext(nc) as tc, ExitStack() as ctx:
        cp = ctx.enter_context(tc.tile_pool(name="consts", bufs=1))
        sb = {}
        for n in WEIGHT_NAMES:
            sb[n] = cp.tile(list(shapes[n]), wdt[n], tag=n, name=n)
            nc.sync.dma_start(sb[n][:], dram[n][:])
        gidx_sb = cp.tile([16, NT * (S * BT) // 16], dt.int16, tag="gidx")
        nc.sync.dma_start(gidx_sb[:], gidx_d[:])
        oh5 = []
        for slab in range(NSLAB):
            oh = cp.tile([D, BC], BF16, tag=f"oh{slab}", name=f"oh{slab}")
            nc.sync.dma_start(oh[:], oh_d[slab * D:(slab + 1) * D, :])
            oh5.append(oh)
        ohp = []
        for p in range(2):
            ohpt = cp.tile([D, BC], BF16, tag=f"ohp{p}", name=f"ohp{p}")
            nc.sync.dma_start(ohpt[:], ohp_d[p * D:(p + 1) * D, :])
            ohp.append(ohpt)

        # tile pools
        psA = ctx.enter_context(tc.tile_pool(name="psA", bufs=2, space="PSUM"))
        psXD = ctx.enter_context(tc.tile_pool(name="psXD", bufs=3, space="PSUM"))
        psP = ctx.enter_context(tc.tile_pool(name="psP", bufs=3, space="PSUM"))
        xgp = ctx.enter_context(tc.tile_pool(name="xgp", bufs=2))
        rbp = ctx.enter_context(tc.tile_pool(name="rbp", bufs=2))
        g2p = ctx.enter_context(tc.tile_pool(name="g2p", bufs=10))
        xnp = ctx.enter_context(tc.tile_pool(name="xnp", bufs=6))
        pbp = ctx.enter_context(tc.tile_pool(name="pbp", bufs=4))
        gpp = ctx.enter_context(tc.tile_pool(name="gpp", bufs=4))
        gtp = ctx.enter_context(tc.tile_pool(name="gtp", bufs=4))
        hbp = ctx.enter_context(tc.tile_pool(name="hbp", bufs=2))
        sqp = ctx.enter_context(tc.tile_pool(name="sqp", bufs=2))
        smp = ctx.enter_context(tc.tile_pool(name="smp", bufs=2))
        accp = ctx.enter_context(tc.tile_pool(name="accp", bufs=2))
        otp = ctx.enter_context(tc.tile_pool(name="otp", bufs=2))

        # per-tile gather destinations (ring of 2)
        xg_t = ([xgp.tile([D, S * BT], BF16, tag="xg", name=f"xg{_i}")
                 for _i in range(NT)] if USE_DMA_GATHER else None)

        def issue_gather(i):
            nc.gpsimd.dma_gather(
                out_ap=xg_t[i][:].rearrange("p (e n) -> p e n", e=1),
                in_ap=dram["emb"][:],
                idxs_ap=gidx_sb[:, i * NIC:(i + 1) * NIC],
                num_idxs=S * BT,
                num_idxs_reg=S * BT,
                elem_size=D,
                transpose=True,
            )

        # gather tiles 0/1 up front, 2/3 mid-stream; the gpsimd ucode
        # library must swap between mlp (dma_gather) and standard
        # (tensor_tensor) around each gather batch.
        if USE_DMA_GATHER:
            nc.gpsimd.load_library(library_config.mlp)
            issue_gather(0)
            issue_gather(1)
            nc.gpsimd.load_library(library_config.standard)

        for i in range(NT):
            bs = bass.ts(i, BT)
            if USE_DMA_GATHER and i == 1:
                nc.gpsimd.load_library(library_config.mlp)
                issue_gather(2)
                issue_gather(3)
                nc.gpsimd.load_library(library_config.standard)

            # ---- ms (mean-square pre-normalizer) from packed one-hots
            ms_ps = psA.tile([D, BT], F32, tag="sm")
            for p in range(2):
                nc.tensor.matmul(ms_ps[:], sb["stm"][:], ohp[p][:, bs],
                                 start=(p == 0), stop=(p == 1))
            sqt = smp.tile([D, BT], F32, tag="sqt")
            nc.scalar.activation(sqt[:], ms_ps[:], AF.Sqrt,
                                 bias=sb["epsb"][:, 0:1], scale=1.0 / S)
            rf = smp.tile([D, BT], F32, tag="rf")
            nc.vector.reciprocal_approx_fast(rf[:], sqt[:])
            rb = rbp.tile([D, BT], BF16, tag="rb")
            nc.vector.tensor_copy(rb[:], rf[:])

            # ---- xsum = sum_s lw[s]*x[b,s,:] straight from slabs
            xs_ps = psA.tile([D, BT], F32, tag="sm")
            for p in range(2):
                nc.tensor.matmul(xs_ps[:], sb["stx"][:, bass.ts(p, D)],
                                 ohp[p][:, bs], start=(p == 0), stop=(p == 1))
            xsum_sb = smp.tile([D, BT], F32, tag="xsum")
            nc.scalar.copy(xsum_sb[:], xs_ps[:])

            # ---- gate coefficients g5[gi]: t = 4gi+q at rows 32q+v
            g5 = []
            for gi in range(NSLAB):
                G_ps = psA.tile([D, BT], F32, tag="sm")
                for p in range(2):
                    nc.tensor.matmul(G_ps[:], sb["stg"][:, bass.ts(gi * 2 + p, D)],
                                     ohp[p][:, bs], start=(p == 0), stop=(p == 1))
                gsb = g2p.tile([D, BT], BF16, tag="g2")
                nc.scalar.copy(gsb[:], G_ps[:])
                g5.append(gsb)

            # ---- per-s pipeline
            hb = hbp.tile([D, S * BT], BF16, tag="hb")
            sqb = sqp.tile([D, S * BT], BF16, tag="sqb")
            for s in range(S):
                ss = bass.ts(s, BT)
                xn = xnp.tile([D, BT], BF16, tag="xn")
                if USE_DMA_GATHER:
                    nc.vector.tensor_mul(xn[:], xg_t[i][:, ss], rb[:])
                else:
                    slab, jj = divmod(s, 4)
                    jb = 32 * jj
                    xgp_ps = psP.tile([D, BT], F32, tag="P")
                    nc.tensor.matmul(xgp_ps[:], sb["emb4g"][jb:jb + V, :],
                                     oh5[slab][jb:jb + V, bs],
                                     start=True, stop=True, tile_position=(jb, 0))
                    nc.vector.tensor_mul(xn[:], xgp_ps[:], rb[:])
                xd = psXD.tile([D, BT], F32, tag="xd")
                nc.tensor.matmul(xd[:], sb["kmat"][:, bass.ts(s, D)], xn[:],
                                 start=True, stop=True)
                gi, q = divmod(s, 4)
                base = 32 * q
                P_ps = psP.tile([D, BT], F32, tag="P")
                nc.tensor.matmul(P_ps[:], sb["emb4g"][base:base + V, :],
                                 g5[gi][base:base + V, :],
                                 start=True, stop=True, tile_position=(base, 0))
                Pb = pbp.tile([D, BT], BF16, tag="Pb")
                nc.scalar.copy(Pb[:], P_ps[:])
                gpre = gpp.tile([D, BT], BF16, tag="gpre")
                nc.vector.tensor_mul(gpre[:], Pb[:], rb[:])
                gt = gtp.tile([D, BT], BF16, tag="gt")
                nc.scalar.activation(gt[:], gpre[:], AFg,
                                     bias=sb["gbb"][:, s:s + 1], scale=1.0)
                hs = hb[:, ss]
                nc.vector.scalar_tensor_tensor(hs, xd[:], sb["lpbsb"][:, s:s + 1],
                                               gt[:], op0=ALU.add, op1=ALU.mult)
                nc.scalar.activation(sqb[:, ss], hs, AF.Square,
                                     bias=sb["zerob"][:, 0:1],
                                     scale=sb["invlpb"][:, s:s + 1])

            # ---- o2/m2 via f32 add chains on GPSIMD (SBUF-only engine)
            o2_sb = accp.tile([D, BT], F32, tag="o2")
            nc.gpsimd.tensor_add(o2_sb[:], hb[:, bass.ts(0, BT)],
                                 hb[:, bass.ts(1, BT)])
            for s in range(2, S):
                nc.gpsimd.tensor_add(o2_sb[:], o2_sb[:], hb[:, bass.ts(s, BT)])
            m2_sb = accp.tile([D, BT], F32, tag="m2")
            nc.gpsimd.tensor_add(m2_sb[:], sqb[:, bass.ts(0, BT)],
                                 sqb[:, bass.ts(1, BT)])
            for s in range(2, S):
                nc.gpsimd.tensor_add(m2_sb[:], m2_sb[:], sqb[:, bass.ts(s, BT)])

            # ---- final combine: out = xsum + rsqrt(m2/S+eps)*o2
            sq2 = smp.tile([D, BT], F32, tag="sq2")
            nc.scalar.activation(sq2[:], m2_sb[:], AF.Sqrt,
                                 bias=sb["epsb"][:, 0:1], scale=1.0 / S)
            rr = smp.tile([D, BT], F32, tag="rr")
            nc.vector.reciprocal_approx_fast(rr[:], sq2[:])
            t1 = smp.tile([D, BT], F32, tag="t1")
            nc.vector.tensor_mul(t1[:], o2_sb[:], rr[:])
            res = otp.tile([D, BT], BF16, tag="res")
            with nc.allow_low_precision(reason="bf16 out within 2e-2 tolerance"):
                nc.vector.tensor_add(res[:], t1[:], xsum_sb[:])
            nc.sync.dma_start(out_d[:, bs], res[:])
    nc.compile()
    return nc


def make_in_maps(inputs):
    import ml_dtypes
    tokens = np.ascontiguousarray(np.asarray(inputs["tokens"], np.int32))
    der = _derived(inputs)
    der = {k: (v.astype(ml_dtypes.bfloat16) if k in BF16_SET else v)
           for k, v in der.items()}
    in_maps = []
    for c in range(NCORES):
        tk = tokens[c * BC:(c + 1) * BC]  # [BC, S]
        # gather indices: tile i, s-major order, wrapped 16 partitions
        gidx = np.empty((16, NT * (S * BT) // 16), np.int16)
        for i in range(NT):
            arr = tk[i * BT:(i + 1) * BT, :].T.reshape(-1)  # [S*BT] s-major
            gidx[:, i * NIC:(i + 1) * NIC] = arr.reshape(-1, 16).T
        # one-hot slabs: oh5[slab*128 + 32j + v, b] = [tokens[b, 4slab+j] == v]
        oh = np.zeros((NSLAB * D, BC), np.float32)
        ohp = np.zeros((2 * D, BC), np.float32)
        for s in range(S):
            slab, j = divmod(s, 4)
            rows = slab * D + 32 * j + tk[:, s]          # [BC] row per example
            oh[rows, np.arange(BC)] = 1.0
            p, sp = divmod(s, 10)
            prow = p * D + 12 * sp + tk[:, s]
            ohp[prow, np.arange(BC)] = 1.0
        m = {"gidx": gidx, "oh5": oh.astype(ml_dtypes.bfloat16),
             "ohp": ohp.astype(ml_dtypes.bfloat16)}
        m.update(der)
        in_maps.append(m)
    return in_maps


def kernel(**inputs):
    from concourse.bass_utils import run_bass_kernel_spmd

    nc = build_program(act_silu=True)
    in_maps = make_in_maps(inputs)
    res = run_bass_kernel_spmd(nc, in_maps, list(range(NCORES)))
    outs = [np.asarray(res.results[c]["out"]).astype(np.float32).T
            for c in range(NCORES)]
    return np.ascontiguousarray(
        np.concatenate(outs, axis=0).reshape(B, 1, D))
